# revision 2
# baseline (speedup 1.0000x reference)
"""Trainium2 Bass kernel for nn_DeformableDynamicGather1D — v3 (window gather).

8 cores = 4 batches x 2 query-halves; per core Q=4096 queries, feat [256,4096].

Design: instead of one dma_gather per tap (6 row-pair gathers/query = 24.5k
idxs/core; Q7 desc-gen at ~9ns/idx was the 200us+ bottleneck), gather per
query ONE 12-row window [b..b+11], b = clip(i0a-5, 0, L-12), which provably
contains every deform tap's bilinear row pair (offsets are bounded:
|off| <= 2*2+0.5 = 4.5 px), plus ONE exact anchor row-pair for the MLP.
=> 2 idxs/query (8.2k/core), desc-gen ~87us.

All feature data is bf16; featT [L, C] bf16 is pre-transposed on the host
(pure layout prep). Gathers are DRAM-source, transpose=False (the only mode
this runtime supports; <=1024 idxs/call), with elem_step=C so the 12-row
window is one 6KB contiguous read.

Combine is query-major: out[q] = sum_j a_j(q) * W[q, j*256:(j+1)*256], where
a_j scatters c0/c1 of the 5 taps into the 12 window rows (built with an
iota/is_equal one-hot + tensor_reduce over taps). The 12 j-FMAs per g-column
are split DVE (j<6: scalar_tensor_tensor FMA into bf16 acc) / ACT+PE (j>=6:
ACT per-partition-scale mult then identity-matmul PSUM accumulate), keeping
both engines ~60us. MLP runs in bf16 (PE). Scalar stage is f32 query-major.
"""
import os
import sys

for _p in ("/opt/trn_rl_repo", "/root/.axon_site/_ro/trn_rl_repo"):
    if os.path.isdir(_p) and _p not in sys.path:
        sys.path.append(_p)

import numpy as np
import concourse.bass as bass
import concourse.bacc as bacc
import concourse.tile as tile
from concourse import mybir
from concourse.bass import AP
from concourse.masks import make_identity

F32 = mybir.dt.float32
BF16 = mybir.dt.bfloat16
I16 = mybir.dt.int16
I32 = mybir.dt.int32
Act = mybir.ActivationFunctionType
Alu = mybir.AluOpType

P = 128          # partitions
G = 32           # q = g*128 + p
Q = P * G        # 4096 queries per core
C = 256          # channels
L = 4096         # feat length
H = 64           # hidden
K = 5            # taps
W = 12           # window rows
NCORES = 8
B, N = 4, 8192   # full problem
NIA = 1024       # anchor idxs per call
NIW = 512        # window idxs per call
NJD = 6          # j's combined on DVE; rest on ACT+PE

IXSCALE = np.float32(float(L - 1))          # 4095
DXSCALE = np.float32(2.0 / max(L - 1, 1))   # reference scale_x


def _bc_in(ap2d: AP, inner: int) -> AP:
    """[p, n] -> [p, n, inner] stride-0 inner dim."""
    return AP(tensor=ap2d.tensor, offset=ap2d.offset,
              ap=[*ap2d.ap, [0, inner]])


def _bc_mid(ap2d: AP, mid: int) -> AP:
    """[p, n] -> [p, mid, n] stride-0 middle dim."""
    return AP(tensor=ap2d.tensor, offset=ap2d.offset,
              ap=[ap2d.ap[0], [0, mid], ap2d.ap[1]])


def build_program():
    nc = bacc.Bacc("TRN2", target_bir_lowering=False, debug=False,
                   num_devices=NCORES)

    featT = nc.dram_tensor("featT", [L, C], BF16, kind="ExternalInput")
    coords = nc.dram_tensor("coords", [Q], F32, kind="ExternalInput")
    cellv = nc.dram_tensor("cellv", [Q], F32, kind="ExternalInput")
    w1a0 = nc.dram_tensor("w1a0", [128, H], BF16, kind="ExternalInput")
    w1a1 = nc.dram_tensor("w1a1", [128, H], BF16, kind="ExternalInput")
    wxc = nc.dram_tensor("wxc", [2, H], BF16, kind="ExternalInput")
    b1c = nc.dram_tensor("b1c", [H, 1], F32, kind="ExternalInput")
    wr1 = nc.dram_tensor("wr1", [H, H], BF16, kind="ExternalInput")
    brc = nc.dram_tensor("brc", [H, 1], F32, kind="ExternalInput")
    w3aug = nc.dram_tensor("w3aug", [H + 1, 12], BF16, kind="ExternalInput")
    base128 = nc.dram_tensor("base128", [P, K], F32, kind="ExternalInput")
    sel8 = nc.dram_tensor("sel8", [P, 8 * 128], F32, kind="ExternalInput")
    iota12 = nc.dram_tensor("iota12", [P, W], F32, kind="ExternalInput")
    out = nc.dram_tensor("out", [Q, C], BF16, kind="ExternalOutput")

    with tile.TileContext(nc) as tc:
        _body(nc, tc, featT, coords, cellv, w1a0, w1a1, wxc, b1c, wr1, brc,
              w3aug, base128, sel8, iota12, out)
    nc.compile()
    return nc


def _body(nc, tc, featT, coords, cellv, w1a0, w1a1, wxc, b1c, wr1, brc,
          w3aug, base128, sel8, iota12, out):
    import contextlib
    ctx = contextlib.ExitStack()
    with ctx:
        const = ctx.enter_context(tc.tile_pool(name="const", bufs=1))
        persist = ctx.enter_context(tc.tile_pool(name="persist", bufs=1))
        sc = ctx.enter_context(tc.tile_pool(name="scal", bufs=1))
        ga = ctx.enter_context(tc.tile_pool(name="ga", bufs=2))
        gw = ctx.enter_context(tc.tile_pool(name="gw", bufs=2))
        fab = ctx.enter_context(tc.tile_pool(name="fab", bufs=3))
        tmu = ctx.enter_context(tc.tile_pool(name="tmu", bufs=3))
        mlpb = ctx.enter_context(tc.tile_pool(name="mlpb", bufs=2))
        pst = ctx.enter_context(tc.tile_pool(name="pst", bufs=2, space="PSUM"))
        psmm = ctx.enter_context(tc.tile_pool(name="psmm", bufs=2, space="PSUM"))
        psl3 = ctx.enter_context(tc.tile_pool(name="psl3", bufs=1, space="PSUM"))
        psac = ctx.enter_context(tc.tile_pool(name="psac", bufs=2, space="PSUM"))

        identf = const.tile([P, P], F32)
        make_identity(nc, identf[:])
        identb = const.tile([P, P], BF16)
        nc.vector.tensor_copy(out=identb[:], in_=identf[:])

        # ---- persistent state ----
        rinT0 = persist.tile([P, Q], BF16)
        rinT1 = persist.tile([P, Q], BF16)
        xc = persist.tile([2, Q], BF16)
        h_sb = persist.tile([H, Q], BF16)
        gaug = persist.tile([H + 1, Q], BF16)
        out3 = persist.tile([P, G, 12], F32)
        acc = persist.tile([P, G, C], BF16)

        # ---- weights / constants ----
        w1a0_sb = const.tile([128, H], BF16)
        w1a1_sb = const.tile([128, H], BF16)
        wxc_sb = const.tile([2, H], BF16)
        b1_sb = const.tile([H, 1], F32)
        wr1_sb = const.tile([H, H], BF16)
        br_sb = const.tile([H, 1], F32)
        w3_sb = const.tile([H + 1, 12], BF16)
        base_sb = const.tile([P, K], F32)
        sel_sb = const.tile([P, 8 * 128], F32)
        iota_sb = const.tile([P, W], F32)
        for dst, src_t in ((w1a0_sb, w1a0), (w1a1_sb, w1a1), (wxc_sb, wxc),
                           (b1_sb, b1c), (wr1_sb, wr1), (br_sb, brc),
                           (w3_sb, w3aug), (base_sb, base128), (sel_sb, sel8),
                           (iota_sb, iota12)):
            nc.sync.dma_start(out=dst[:], in_=src_t.ap())

        xcf = sc.tile([2, Q], F32)
        nc.sync.dma_start(out=xcf[0:1, :], in_=coords.ap().rearrange(
            "(a q) -> a q", a=1))
        nc.sync.dma_start(out=xcf[1:2, :], in_=cellv.ap().rearrange(
            "(a q) -> a q", a=1))
        nc.scalar.copy(out=xc[:], in_=xcf[:])

        xq = persist.tile([P, G], F32)
        nc.sync.dma_start(
            out=xq[:],
            in_=AP(tensor=coords.ap().tensor, offset=0, ap=[[1, P], [P, G]]))

        # gather sources over featT (bf16): row-pair and 12-row window
        gsrcA = AP(tensor=featT.ap().tensor, offset=0,
                   ap=[[C, L - 1], [1, 2 * C]])
        gsrcW = AP(tensor=featT.ap().tensor, offset=0,
                   ap=[[C, L - W + 1], [1, W * C]])

        def wrapped_idx(vf32_ap, nk, tag):
            wrep = sc.tile([P, nk, Q // 16], I16, tag=tag + "_wrep")
            for a in range(8):
                psw = psl3.tile([P, nk * G], F32, tag="pswrap", space="PSUM")
                nc.tensor.matmul(
                    out=psw[:], lhsT=sel_sb[:, a * 128:(a + 1) * 128],
                    rhs=vf32_ap, start=True, stop=True)
                dst = AP(tensor=wrep[:].tensor, offset=wrep[:].offset + a,
                         ap=[wrep[:].ap[0], [Q // 16, nk], [8, G]])
                srcap = AP(tensor=psw[:].tensor, offset=psw[:].offset,
                           ap=[psw[:].ap[0], [1, nk], [nk, G]])
                nc.vector.tensor_copy(out=dst, in_=srcap)
            return wrep

        # ---- anchor indices + window base ----
        ixf = sc.tile([P, G], F32)
        nc.vector.tensor_scalar(out=ixf[:], in0=xq[:], scalar1=1.0,
                                scalar2=0.5, op0=Alu.add, op1=Alu.mult)
        nc.vector.tensor_scalar(out=ixf[:], in0=ixf[:], scalar1=float(IXSCALE),
                                scalar2=0.0, op0=Alu.mult, op1=Alu.max)
        nc.vector.tensor_scalar(out=ixf[:], in0=ixf[:], scalar1=float(IXSCALE),
                                scalar2=None, op0=Alu.min)
        fraca = sc.tile([P, G], F32)
        i0fa = sc.tile([P, G], F32)
        ti_a = sc.tile([P, G], I32)
        nc.vector.tensor_copy(out=ti_a[:], in_=ixf[:])
        nc.vector.tensor_copy(out=i0fa[:], in_=ti_a[:])
        gt_a = sc.tile([P, G], F32)
        nc.vector.tensor_tensor(out=gt_a[:], in0=i0fa[:], in1=ixf[:],
                                op=Alu.is_gt)
        nc.vector.tensor_tensor(out=i0fa[:], in0=i0fa[:], in1=gt_a[:],
                                op=Alu.subtract)
        nc.vector.tensor_scalar(out=i0fa[:], in0=i0fa[:], scalar1=float(L - 2),
                                scalar2=None, op0=Alu.min)
        nc.vector.tensor_tensor(out=fraca[:], in0=ixf[:], in1=i0fa[:],
                                op=Alu.subtract)
        bwin = sc.tile([P, G], F32)
        nc.vector.tensor_scalar(out=bwin[:], in0=i0fa[:], scalar1=-5.0,
                                scalar2=0.0, op0=Alu.add, op1=Alu.max)
        nc.vector.tensor_scalar(out=bwin[:], in0=bwin[:],
                                scalar1=float(L - W), scalar2=None,
                                op0=Alu.min)

        wrapA = wrapped_idx(i0fa[:], 1, "wa")
        wrapW = wrapped_idx(bwin[:], 1, "ww")

        # ---- anchor gathers + lerp + transpose -> rinT ----
        for ch in range(Q // NIA):
            Ga = ga.tile([P, NIA // P, 2 * C], BF16, tag="ga")
            nc.gpsimd.dma_gather(
                out_ap=Ga[:], in_ap=gsrcA,
                idxs_ap=wrapA[:, 0, ch * (NIA // 16):(ch + 1) * (NIA // 16)],
                num_idxs=NIA, num_idxs_reg=NIA, elem_size=2 * C, elem_step=C)
            for gi in range(NIA // P):
                g = ch * (NIA // P) + gi
                d = fab.tile([P, C], BF16, tag="dl")
                nc.vector.tensor_tensor(out=d[:], in0=Ga[:, gi, C:2 * C],
                                        in1=Ga[:, gi, 0:C], op=Alu.subtract)
                fa = fab.tile([P, C], BF16, tag="fa")
                nc.vector.scalar_tensor_tensor(
                    out=fa[:], in0=d[:], scalar=fraca[:, g:g + 1],
                    in1=Ga[:, gi, 0:C], op0=Alu.mult, op1=Alu.add)
                for hh in range(2):
                    tpa = pst.tile([P, P], BF16, tag="tpsum", space="PSUM")
                    nc.tensor.transpose(out=tpa[:],
                                        in_=fa[:, hh * 128:(hh + 1) * 128],
                                        identity=identb[:])
                    rdst = (rinT0 if hh == 0 else rinT1)
                    nc.scalar.copy(out=rdst[:, g * 128:(g + 1) * 128],
                                   in_=tpa[:])

        # ---- MLP ----
        nc.vector.memset(gaug[H:H + 1, :], 1.0)
        for n in range(8):
            sl = slice(n * 512, (n + 1) * 512)
            ps1 = psmm.tile([H, 512], F32, tag="ps1", space="PSUM")
            nc.tensor.matmul(out=ps1[:], lhsT=w1a0_sb[:], rhs=rinT0[:, sl],
                             start=True, stop=False)
            nc.tensor.matmul(out=ps1[:], lhsT=w1a1_sb[:], rhs=rinT1[:, sl],
                             start=False, stop=False)
            nc.tensor.matmul(out=ps1[:], lhsT=wxc_sb[:], rhs=xc[:, sl],
                             start=False, stop=True)
            tmp = mlpb.tile([H, 512], F32, tag="mlptmp")
            nc.scalar.activation(out=tmp[:], in_=ps1[:], func=Act.Identity,
                                 bias=b1_sb[:, :], scale=1.0)
            nc.vector.scalar_tensor_tensor(out=h_sb[:, sl], in0=tmp[:],
                                           scalar=0.2, in1=tmp[:],
                                           op0=Alu.mult, op1=Alu.max)
        for n in range(8):
            sl = slice(n * 512, (n + 1) * 512)
            ps2 = psmm.tile([H, 512], F32, tag="ps1", space="PSUM")
            nc.tensor.matmul(out=ps2[:], lhsT=wr1_sb[:], rhs=h_sb[:, sl],
                             start=True, stop=True)
            tmp2 = mlpb.tile([H, 512], F32, tag="mlptmp")
            nc.scalar.activation(out=tmp2[:], in_=ps2[:], func=Act.Identity,
                                 bias=br_sb[:, :], scale=1.0)
            nc.vector.scalar_tensor_tensor(out=gaug[0:H, sl], in0=tmp2[:],
                                           scalar=0.2, in1=tmp2[:],
                                           op0=Alu.mult, op1=Alu.max)
        for g in range(G):
            ps3 = psl3.tile([P, 12], F32, tag="ps3", space="PSUM")
            nc.tensor.matmul(out=ps3[:], lhsT=gaug[:, g * 128:(g + 1) * 128],
                             rhs=w3_sb[:], start=True, stop=True)
            nc.scalar.copy(out=out3[:, g, :], in_=ps3[:])

        # ---- scalar stage ----
        def softplus(dst, src_ap):
            a = sc.tile([P, G], F32, tag="sp_a")
            nc.scalar.activation(out=a[:], in_=src_ap, func=Act.Abs)
            e = sc.tile([P, G], F32, tag="sp_e")
            nc.scalar.activation(out=e[:], in_=a[:], func=Act.Exp, scale=-1.0)
            lg = sc.tile([P, G], F32, tag="sp_l")
            nc.scalar.activation(out=lg[:], in_=e[:], func=Act.Ln, bias=1.0,
                                 scale=1.0)
            m = sc.tile([P, G], F32, tag="sp_m")
            nc.vector.tensor_scalar(out=m[:], in0=src_ap, scalar1=0.0,
                                    scalar2=None, op0=Alu.max)
            nc.vector.tensor_tensor(out=dst, in0=lg[:], in1=m[:], op=Alu.add)

        r_t = sc.tile([P, G], F32)
        softplus(r_t[:], out3[:, :, 0])
        nc.vector.tensor_scalar(out=r_t[:], in0=r_t[:], scalar1=0.3,
                                scalar2=2.0, op0=Alu.add, op1=Alu.min)
        sg_t = sc.tile([P, G], F32)
        softplus(sg_t[:], out3[:, :, 1])
        nc.vector.tensor_scalar(out=sg_t[:], in0=sg_t[:], scalar1=0.5,
                                scalar2=3.0, op0=Alu.add, op1=Alu.min)
        s2 = sc.tile([P, G], F32)
        nc.vector.tensor_tensor(out=s2[:], in0=sg_t[:], in1=sg_t[:],
                                op=Alu.mult)
        nc.vector.tensor_scalar(out=s2[:], in0=s2[:], scalar1=4.0,
                                scalar2=1e-8, op0=Alu.mult, op1=Alu.add)
        rs = sc.tile([P, G], F32)
        nc.vector.reciprocal(out=rs[:], in_=s2[:])

        res_t = sc.tile([P, G * K], F32)
        nc.scalar.activation(out=res_t[:], in_=out3[:, :, 2:7], func=Act.Tanh)
        gate_t = sc.tile([P, G * K], F32)
        nc.scalar.activation(out=gate_t[:], in_=out3[:, :, 7:12],
                             func=Act.Sigmoid)

        off_t = sc.tile([P, G * K], F32)
        nc.vector.tensor_tensor(out=off_t[:], in0=_bc_in(r_t[:], K),
                                in1=_bc_mid(base_sb[:], G), op=Alu.mult)
        nc.vector.scalar_tensor_tensor(out=off_t[:], in0=res_t[:], scalar=0.5,
                                       in1=off_t[:], op0=Alu.mult, op1=Alu.add)
        dix = sc.tile([P, G * K], F32)
        nc.vector.scalar_tensor_tensor(out=dix[:], in0=off_t[:],
                                       scalar=float(DXSCALE),
                                       in1=_bc_in(xq[:], K),
                                       op0=Alu.mult, op1=Alu.add)
        nc.vector.tensor_scalar(out=dix[:], in0=dix[:], scalar1=1.0,
                                scalar2=0.5, op0=Alu.add, op1=Alu.mult)
        nc.vector.tensor_scalar(out=dix[:], in0=dix[:], scalar1=float(IXSCALE),
                                scalar2=0.0, op0=Alu.mult, op1=Alu.max)
        nc.vector.tensor_scalar(out=dix[:], in0=dix[:], scalar1=float(IXSCALE),
                                scalar2=None, op0=Alu.min)
        fracd = sc.tile([P, G * K], F32)
        i0fd = sc.tile([P, G * K], F32)
        ti_d = sc.tile([P, G * K], I32)
        nc.vector.tensor_copy(out=ti_d[:], in_=dix[:])
        nc.vector.tensor_copy(out=i0fd[:], in_=ti_d[:])
        gt_d = sc.tile([P, G * K], F32)
        nc.vector.tensor_tensor(out=gt_d[:], in0=i0fd[:], in1=dix[:],
                                op=Alu.is_gt)
        nc.vector.tensor_tensor(out=i0fd[:], in0=i0fd[:], in1=gt_d[:],
                                op=Alu.subtract)
        nc.vector.tensor_scalar(out=i0fd[:], in0=i0fd[:], scalar1=float(L - 2),
                                scalar2=None, op0=Alu.min)
        nc.vector.tensor_tensor(out=fracd[:], in0=dix[:], in1=i0fd[:],
                                op=Alu.subtract)

        o2 = sc.tile([P, G * K], F32)
        nc.vector.tensor_tensor(out=o2[:], in0=off_t[:], in1=off_t[:],
                                op=Alu.mult)
        nc.vector.tensor_tensor(out=o2[:], in0=o2[:], in1=_bc_in(rs[:], K),
                                op=Alu.mult)
        w_t = sc.tile([P, G * K], F32)
        nc.scalar.activation(out=w_t[:], in_=o2[:], func=Act.Exp, scale=-0.5)
        nc.vector.tensor_tensor(out=w_t[:], in0=w_t[:], in1=gate_t[:],
                                op=Alu.mult)
        wsum = sc.tile([P, G], F32)
        w_v = w_t[:].rearrange("p (g k) -> p g k", k=K)
        nc.vector.tensor_reduce(out=wsum[:], in_=w_v, axis=mybir.AxisListType.X,
                                op=Alu.add)
        nc.vector.tensor_scalar(out=wsum[:], in0=wsum[:], scalar1=1e-8,
                                scalar2=None, op0=Alu.add)
        rn = sc.tile([P, G], F32)
        nc.vector.reciprocal(out=rn[:], in_=wsum[:])
        wn = sc.tile([P, G * K], F32)
        nc.vector.tensor_tensor(out=wn[:], in0=w_t[:], in1=_bc_in(rn[:], K),
                                op=Alu.mult)
        c1 = sc.tile([P, G * K], F32)
        nc.vector.tensor_tensor(out=c1[:], in0=wn[:], in1=fracd[:],
                                op=Alu.mult)
        c0 = sc.tile([P, G * K], F32)
        nc.vector.tensor_tensor(out=c0[:], in0=wn[:], in1=c1[:],
                                op=Alu.subtract)

        # j0 = i0fd - bwin (integers in [0, W-2])
        j0f = sc.tile([P, G * K], F32)
        nc.vector.tensor_tensor(out=j0f[:], in0=i0fd[:],
                                in1=_bc_in(bwin[:], K), op=Alu.subtract)

        # ---- a_j [p, g, W]: scatter c0 at j0, c1 at j0+1 ----
        oh = sc.tile([P, G * K, W], F32)       # [p, (g k), j]
        nc.vector.tensor_tensor(out=oh[:], in0=_bc_in(j0f[:], W),
                                in1=_bc_mid(iota_sb[:], G * K),
                                op=Alu.is_equal)
        m1 = sc.tile([P, G * K, W], F32)
        nc.vector.tensor_tensor(out=m1[:], in0=oh[:], in1=_bc_in(c1[:], W),
                                op=Alu.mult)
        nc.vector.tensor_tensor(out=oh[:], in0=oh[:], in1=_bc_in(c0[:], W),
                                op=Alu.mult)
        aj = sc.tile([P, G, W], F32)
        a1 = sc.tile([P, G, W], F32)
        # reduce over k: view [p, g, j, k]
        ohv = AP(tensor=oh[:].tensor, offset=oh[:].offset,
                 ap=[oh[:].ap[0], [K * W, G], [1, W], [W, K]])
        m1v = AP(tensor=m1[:].tensor, offset=m1[:].offset,
                 ap=[m1[:].ap[0], [K * W, G], [1, W], [W, K]])
        nc.vector.tensor_reduce(out=aj[:], in_=ohv, axis=mybir.AxisListType.X,
                                op=Alu.add)
        nc.vector.tensor_reduce(out=a1[:], in_=m1v, axis=mybir.AxisListType.X,
                                op=Alu.add)
        # aj[:, g, j] += a1[:, g, j-1]
        ajs = AP(tensor=aj[:].tensor, offset=aj[:].offset + 1,
                 ap=[aj[:].ap[0], [W, G], [1, W - 1]])
        a1s = AP(tensor=a1[:].tensor, offset=a1[:].offset,
                 ap=[a1[:].ap[0], [W, G], [1, W - 1]])
        nc.vector.tensor_tensor(out=ajs, in0=ajs, in1=a1s, op=Alu.add)

        # ---- window gathers + combine ----
        for ch in range(Q // NIW):
            Gw = gw.tile([P, NIW // P, W * C], BF16, tag="gw")
            nc.gpsimd.dma_gather(
                out_ap=Gw[:], in_ap=gsrcW,
                idxs_ap=wrapW[:, 0, ch * (NIW // 16):(ch + 1) * (NIW // 16)],
                num_idxs=NIW, num_idxs_reg=NIW, elem_size=W * C, elem_step=C)
            for gi in range(NIW // P):
                g = ch * (NIW // P) + gi
                accg = acc[:, g, :]
                psg = psac.tile([P, C], F32, tag="psac", space="PSUM")
                for j in range(W):
                    wslice = Gw[:, gi, j * C:(j + 1) * C]
                    ajs_g = aj[:, g, j:j + 1]
                    if j == 0:
                        nc.vector.tensor_scalar(out=accg, in0=wslice,
                                                scalar1=ajs_g, scalar2=None,
                                                op0=Alu.mult)
                    elif j < NJD:
                        nc.vector.scalar_tensor_tensor(
                            out=accg, in0=wslice, scalar=ajs_g, in1=accg,
                            op0=Alu.mult, op1=Alu.add)
                    else:
                        t = tmu.tile([P, C], BF16, tag="tm")
                        nc.scalar.activation(out=t[:], in_=wslice,
                                             func=Act.Identity, scale=ajs_g)
                        nc.tensor.matmul(out=psg[:], lhsT=identb[:], rhs=t[:],
                                         start=(j == NJD), stop=(j == W - 1))
                nc.vector.tensor_tensor(out=accg, in0=accg, in1=psg[:],
                                        op=Alu.add)

        nc.sync.dma_start(
            out=out.ap().rearrange("(g p) c -> p g c", p=P), in_=acc[:])


_PROGRAM = None


def _get_program():
    global _PROGRAM
    if _PROGRAM is None:
        _PROGRAM = build_program()
    return _PROGRAM


def make_in_maps(feat_1d, coords_1d, cell_1d, W1, b1, Wr, br, W3, b3):
    import ml_dtypes
    f32 = np.float32
    bf16 = ml_dtypes.bfloat16
    W1 = np.asarray(W1, f32)
    wr1 = np.asarray(Wr, f32) + np.eye(H, dtype=f32)
    w3aug = np.concatenate([np.asarray(W3, f32),
                            np.asarray(b3, f32).reshape(1, 12)], axis=0)
    base = np.array([-2.0, -1.0, 0.0, 1.0, 2.0], f32)
    base128 = np.broadcast_to(base, (P, K)).copy()
    sel = np.zeros((P, 8, 128), f32)
    for a in range(8):
        for m in range(128):
            sel[16 * a + m % 16, a, m] = 1.0
    iota = np.broadcast_to(np.arange(W, dtype=f32), (P, W)).copy()
    shared = {
        "w1a0": np.ascontiguousarray(W1[0:128]).astype(bf16),
        "w1a1": np.ascontiguousarray(W1[128:256]).astype(bf16),
        "wxc": np.ascontiguousarray(W1[256:258]).astype(bf16),
        "b1c": np.asarray(b1, f32).reshape(H, 1).copy(),
        "wr1": wr1.astype(bf16),
        "brc": np.asarray(br, f32).reshape(H, 1).copy(),
        "w3aug": w3aug.astype(bf16),
        "base128": base128,
        "sel8": sel.reshape(P, 8 * 128),
        "iota12": iota,
    }
    in_maps = []
    for core in range(NCORES):
        b = core // 2
        s = core % 2
        sl = slice(s * Q, (s + 1) * Q)
        ft = np.ascontiguousarray(np.asarray(feat_1d[b], f32).T)  # [L, C]
        in_maps.append({
            "featT": ft.astype(bf16),
            "coords": np.ascontiguousarray(np.asarray(coords_1d[b, sl, 0], f32)),
            "cellv": np.ascontiguousarray(np.asarray(cell_1d[b, sl, 0], f32)),
            **shared,
        })
    return in_maps


def kernel(feat_1d, coords_1d, cell_1d, W1, b1, Wr, br, W3, b3):
    from concourse.bass_utils import run_bass_kernel_spmd
    nc = _get_program()
    in_maps = make_in_maps(feat_1d, coords_1d, cell_1d, W1, b1, Wr, br, W3, b3)
    res = run_bass_kernel_spmd(nc, in_maps, core_ids=list(range(NCORES)))
    outf = np.zeros((B, N, C), np.float32)
    for core in range(NCORES):
        b = core // 2
        s = core % 2
        outf[b, s * Q:(s + 1) * Q, :] = np.asarray(
            res.results[core]["out"], np.float32)
    return outf
